# revision 9
# baseline (speedup 1.0000x reference)
"""Causal self-attention (single head) on 8 Trainium2 NeuronCores.

Sharding: 8 cores = 4 batches x 2 query-tile parity sets. Core c handles
batch (c % 4). Cores 0-3 take query tiles t in {15,13,...,1} (128 rows
each), cores 4-7 take t in {14,12,...,0}. Program iteration i=0..7 uses a
fixed causal extent E(i) = 16-2i k-tiles, so a single SPMD program serves
all cores; even-parity cores waste one fully-masked k-tile per iteration.

Host passes x.T (plus the core's own query columns pre-gathered) and W.T
per core so the device never transposes inputs; matmuls run as float32r.
Softmax skips max-subtraction (scores/32 stay in a safe exp range) and
gets row sums free via the activation accum_out.
"""

import sys

for _p in ("/opt/trn_rl_repo", "/root/.axon_site/_ro/trn_rl_repo"):
    if _p not in sys.path:
        sys.path.append(_p)

import numpy as np

import concourse.bass as bass  # noqa: F401
import concourse.mybir as mybir
import concourse.tile as tile
from concourse import bacc
from concourse.bass_utils import run_bass_kernel_spmd

F32 = mybir.dt.float32
F32R = mybir.dt.float32r

BATCH, SEQ, D, P = 4, 2048, 1024, 1024
N_CORES = 8
QT = 128          # query tile rows
KTL = 128         # key tile
NBLK = 512        # matmul moving free dim
ND = D // 128     # 8 d-tiles
NP = P // 128     # 8 p-tiles
NKT = SEQ // KTL  # 16 k-tiles
NQT = 8           # q-tiles per core
SCALE = 1.0 / float(np.sqrt(P))
NEG = -1e9


def _extent(i):
    return 16 - 2 * i


def _chunks(width):
    out = []
    w = width
    while w >= NBLK:
        out.append(NBLK)
        w -= NBLK
    if w:
        assert w == 256, w
        out.append(256)
    return out


def build_program():
    nc = bacc.Bacc("TRN2", target_bir_lowering=False)

    xT = nc.dram_tensor("xT", [D, SEQ], F32, kind="ExternalInput")
    xq_in = nc.dram_tensor("xqcols", [D, NQT * QT], F32, kind="ExternalInput")
    WqT = nc.dram_tensor("WqT", [D, P], F32, kind="ExternalInput")
    WkT = nc.dram_tensor("WkT", [D, P], F32, kind="ExternalInput")
    WvT = nc.dram_tensor("WvT", [D, P], F32, kind="ExternalInput")
    mask = nc.dram_tensor("mask", [QT, 256], F32, kind="ExternalInput")
    ident_in = nc.dram_tensor("ident", [128, 128], F32, kind="ExternalInput")
    out = nc.dram_tensor("out", [NQT * QT, P], F32, kind="ExternalOutput")

    qt_scr = nc.dram_tensor("qt_scr", [P, NQT * QT], F32)

    # DRAM views with the 128-partition dim innermost-first for tiled DMA
    xT_r = xT.rearrange("(dt dp) s -> dp dt s", dp=128)          # [128, 8, SEQ]
    xq_r = xq_in.rearrange("(dt dp) q -> dp dt q", dp=128)       # [128, 8, 1024]
    qt_scr_r = qt_scr.rearrange("(pt pp) q -> pp pt q", pp=128)  # [128, 8, 1024]

    with tile.TileContext(nc) as tc:
        with tc.tile_pool(name="resident", bufs=1) as resident:
            # --- resident tensors (live the whole kernel) ---
            kt_sb = resident.tile([128, NP, SEQ], F32R)     # K.T  [p, k]
            v_sb = resident.tile([128, NKT, P], F32R)       # V    [k, p]
            mask_sb = resident.tile([QT, 256], F32)
            ident = resident.tile([128, 128], F32R)
            nc.sync.dma_start(out=mask_sb, in_=mask[:, :])
            nc.sync.dma_start(out=ident, in_=ident_in[:, :].bitcast(F32R))

            _phase0(nc, tc, kt_sb, v_sb, WqT, WkT, WvT, xT_r, xq_r, qt_scr_r)
            _phase1(nc, tc, kt_sb, v_sb, mask_sb, ident, qt_scr_r, out)

    nc.compile()
    return nc


def _phase0(nc, tc, kt_sb, v_sb, WqT, WkT, WvT, xT_r, xq_r, qt_scr_r):
    with (
        tc.tile_pool(name="wpool", bufs=1) as wpool,
        tc.tile_pool(name="xstage", bufs=1) as xstage,
        tc.tile_pool(name="stage", bufs=3) as stage,
        tc.tile_pool(name="p0psum", bufs=4, space="PSUM") as p0psum,
    ):
            # --- phase 0a: Q.T -> DRAM scratch (i-ordered columns) ---
            for wh in range(2):  # W column halves (p-tiles 4*wh..4*wh+3)
                wq_h = wpool.tile([128, ND, 4 * 128], F32R, tag="w")
                nc.sync.dma_start(
                    out=wq_h,
                    in_=WqT.rearrange("(dt dp) p -> dp dt p", dp=128)[
                        :, :, wh * 512:(wh + 1) * 512].bitcast(F32R),
                )
                for qg in range(2):
                    xq = xstage.tile([128, ND, NBLK], F32R, tag="xq")
                    nc.sync.dma_start(
                        out=xq,
                        in_=xq_r[:, :, qg * 512:(qg + 1) * 512].bitcast(F32R),
                    )
                    for ptl in range(4):
                        pt = 4 * wh + ptl
                        ps = p0psum.tile([128, NBLK], F32, tag="p0")
                        for d in range(ND):
                            nc.tensor.matmul(
                                ps,
                                wq_h[:, d, ptl * 128:(ptl + 1) * 128],
                                xq[:, d, :],
                                start=(d == 0),
                                stop=(d == ND - 1),
                            )
                        st = stage.tile([128, NBLK], F32R, tag="qst")
                        nc.scalar.copy(st, ps)
                        nc.sync.dma_start(
                            out=qt_scr_r[:, pt, qg * 512:(qg + 1) * 512
                                         ].bitcast(F32R),
                            in_=st,
                        )

            # --- phase 0b+0c: K.T and V (fused over k-blocks, W halves) ---
            for h in range(2):
                wk_h = wpool.tile([128, ND, 4 * 128], F32R, tag="w")
                nc.sync.dma_start(
                    out=wk_h,
                    in_=WkT.rearrange("(dt dp) p -> dp dt p", dp=128)[
                        :, :, h * 512:(h + 1) * 512].bitcast(F32R),
                )
                wv_h = wpool.tile([128, ND, NBLK], F32R, tag="wv")
                nc.sync.dma_start(
                    out=wv_h,
                    in_=WvT.rearrange("(dt dp) p -> dp dt p", dp=128)[
                        :, :, h * 512:(h + 1) * 512].bitcast(F32R),
                )
                for kb in range(SEQ // NBLK):
                    xk = xstage.tile([128, ND, NBLK], F32R, tag="xq")
                    nc.sync.dma_start(
                        out=xk,
                        in_=xT_r[:, :, kb * NBLK:(kb + 1) * NBLK].bitcast(F32R),
                    )
                    # K.T rows for p-tiles in this half
                    for ptl in range(4):
                        pt = 4 * h + ptl
                        ps = p0psum.tile([128, NBLK], F32, tag="p0")
                        for d in range(ND):
                            nc.tensor.matmul(
                                ps,
                                wk_h[:, d, ptl * 128:(ptl + 1) * 128],
                                xk[:, d, :],
                                start=(d == 0),
                                stop=(d == ND - 1),
                            )
                        nc.scalar.copy(kt_sb[:, pt, kb * NBLK:(kb + 1) * NBLK], ps)
                    # V columns for this half: 4 k-tiles per block
                    for j in range(NBLK // KTL):
                        ktile = kb * (NBLK // KTL) + j
                        ps = p0psum.tile([128, NBLK], F32, tag="p0")
                        for d in range(ND):
                            nc.tensor.matmul(
                                ps,
                                xk[:, d, j * 128:(j + 1) * 128],
                                wv_h[:, d, :],
                                start=(d == 0),
                                stop=(d == ND - 1),
                            )
                        nc.vector.tensor_copy(
                            v_sb[:, ktile, h * NBLK:(h + 1) * NBLK], ps
                        )


def _phase1(nc, tc, kt_sb, v_sb, mask_sb, ident, qt_scr_r, out):
    with (
        tc.tile_pool(name="qtp", bufs=2) as qtp,
        tc.tile_pool(name="wrow", bufs=2) as wrow,
        tc.tile_pool(name="small", bufs=4) as small,
        tc.tile_pool(name="outp", bufs=2) as outp,
        tc.tile_pool(name="spsum", bufs=3, space="PSUM") as spsum,
        tc.tile_pool(name="s2psum", bufs=1, space="PSUM") as s2psum,
        tc.tile_pool(name="zpsum", bufs=2, space="PSUM") as zpsum,
        tc.tile_pool(name="tpsum", bufs=2, space="PSUM") as tpsum,
    ):
            for i in range(NQT):
                ext = _extent(i)
                width = ext * KTL
                chunks = _chunks(width)

                qt_sb = qtp.tile([128, NP, QT], F32R, tag="qt")
                nc.sync.dma_start(
                    out=qt_sb,
                    in_=qt_scr_r[:, :, i * QT:(i + 1) * QT].bitcast(F32R),
                )

                # scores chunks in PSUM
                s_ps = []
                off = 0
                for cw in chunks:
                    pool = spsum if cw == NBLK else s2psum
                    ps = pool.tile([QT, cw], F32, tag=f"s{cw}")
                    for pt in range(NP):
                        nc.tensor.matmul(
                            ps,
                            qt_sb[:, pt, :],
                            kt_sb[:, pt, off:off + cw],
                            start=(pt == 0),
                            stop=(pt == NP - 1),
                        )
                    s_ps.append((ps, off, cw))
                    off += cw

                # additive causal mask on the last 256 columns of the row
                last_ps, _, last_w = s_ps[-1]
                nc.vector.tensor_add(
                    last_ps[:, last_w - 256:last_w],
                    last_ps[:, last_w - 256:last_w],
                    mask_sb,
                )

                # exp((s + m) * scale) -> SBUF weights row; row-sums per chunk
                w_sb = wrow.tile([QT, width], F32R, tag="w")
                lparts = small.tile([QT, len(chunks)], F32, tag="lp")
                for ci, (ps, off_c, cw) in enumerate(s_ps):
                    nc.scalar.activation(
                        w_sb[:, off_c:off_c + cw],
                        ps,
                        mybir.ActivationFunctionType.Exp,
                        scale=SCALE,
                        accum_out=lparts[:, ci:ci + 1],
                    )

                lsum = small.tile([QT, 1], F32, tag="ls")
                nc.vector.reduce_sum(lsum, lparts, axis=mybir.AxisListType.X)
                rl = small.tile([QT, 1], F32, tag="rl")
                nc.vector.reciprocal(rl, lsum)

                # AV: transpose each weight block on PE, accumulate Z
                z0 = zpsum.tile([QT, NBLK], F32, tag="z")
                z1 = zpsum.tile([QT, NBLK], F32, tag="z")
                for kt in range(ext):
                    tp = tpsum.tile([128, 128], F32R, tag="tp")
                    nc.tensor.transpose(
                        tp, w_sb[:, kt * 128:(kt + 1) * 128], ident
                    )
                    wT = small.tile([128, 128], F32R, tag="wT")
                    nc.vector.tensor_copy(wT, tp)
                    nc.tensor.matmul(
                        z0, wT, v_sb[:, kt, 0:NBLK],
                        start=(kt == 0), stop=(kt == ext - 1),
                    )
                    nc.tensor.matmul(
                        z1, wT, v_sb[:, kt, NBLK:P],
                        start=(kt == 0), stop=(kt == ext - 1),
                    )

                o_sb = outp.tile([QT, P], F32, tag="o")
                nc.vector.tensor_scalar_mul(o_sb[:, 0:NBLK], z0, rl)
                nc.vector.tensor_scalar_mul(o_sb[:, NBLK:P], z1, rl)
                nc.sync.dma_start(out=out[i * QT:(i + 1) * QT, :], in_=o_sb)


def _tiles_for_core(c):
    """Global 128-row query-tile indices, in program order i=0..7."""
    return [(15 - 2 * i) if c < 4 else (14 - 2 * i) for i in range(NQT)]


def _host_prep(inputs, Wq, Wk, Wv):
    x = np.asarray(inputs, dtype=np.float32)
    WqT = np.ascontiguousarray(np.asarray(Wq, dtype=np.float32).T)
    WkT = np.ascontiguousarray(np.asarray(Wk, dtype=np.float32).T)
    WvT = np.ascontiguousarray(np.asarray(Wv, dtype=np.float32).T)

    qi = np.arange(QT)[:, None]
    ki = np.arange(128)[None, :]
    tri = np.where(qi >= ki, 0.0, NEG).astype(np.float32)
    mask_hi = np.concatenate([np.zeros((QT, 128), np.float32), tri], axis=1)
    mask_lo = np.concatenate([tri, np.full((QT, 128), NEG, np.float32)], axis=1)

    in_maps = []
    xT_cache = {}
    for c in range(N_CORES):
        b = c % 4
        if b not in xT_cache:
            xT_cache[b] = np.ascontiguousarray(x[b].T)  # [D, SEQ]
        xTb = xT_cache[b]
        cols = np.concatenate(
            [xTb[:, t * QT:(t + 1) * QT] for t in _tiles_for_core(c)], axis=1
        )
        in_maps.append({
            "xT": xTb,
            "xqcols": np.ascontiguousarray(cols),
            "WqT": WqT,
            "WkT": WkT,
            "WvT": WvT,
            "mask": mask_hi if c < 4 else mask_lo,
            "ident": np.eye(128, dtype=np.float32),
        })
    return in_maps


def _host_gather(results):
    Z = np.empty((BATCH, SEQ, P), dtype=np.float32)
    for c in range(N_CORES):
        b = c % 4
        o = results[c]["out"]
        for i, t in enumerate(_tiles_for_core(c)):
            Z[b, t * QT:(t + 1) * QT, :] = o[i * QT:(i + 1) * QT, :]
    return Z


_NC_CACHE = None


def kernel(inputs, Wq, Wk, Wv):
    global _NC_CACHE
    if _NC_CACHE is None:
        _NC_CACHE = build_program()
    in_maps = _host_prep(inputs, Wq, Wk, Wv)
    res = run_bass_kernel_spmd(_NC_CACHE, in_maps, list(range(N_CORES)))
    return _host_gather(res.results)


# revision 10
# speedup vs baseline: 1.1875x; 1.1875x over previous
"""Causal self-attention (single head) on 8 Trainium2 NeuronCores.

Sharding: 8 cores = 4 batches x 2 query-tile parity sets. Core c handles
batch (c % 4). Cores 0-3 take query tiles t in {15,13,...,1} (128 rows
each), cores 4-7 take t in {14,12,...,0}. Program iteration i=0..7 uses a
fixed causal extent E(i) = 16-2i k-tiles, so a single SPMD program serves
all cores; even-parity cores waste one fully-masked k-tile per iteration.

Host passes x.T (plus the core's own query columns pre-gathered) and W.T
per core so the device never transposes inputs; matmuls run as float32r.
Softmax skips max-subtraction (scores/32 stay in a safe exp range) and
gets row sums free via the activation accum_out.
"""

import sys

for _p in ("/opt/trn_rl_repo", "/root/.axon_site/_ro/trn_rl_repo"):
    if _p not in sys.path:
        sys.path.append(_p)

import numpy as np

import concourse.bass as bass  # noqa: F401
import concourse.mybir as mybir
import concourse.tile as tile
from concourse import bacc
from concourse.bass_utils import run_bass_kernel_spmd

F32 = mybir.dt.float32
F16 = mybir.dt.float16

BATCH, SEQ, D, P = 4, 2048, 1024, 1024
N_CORES = 8
QT = 128          # query tile rows
KTL = 128         # key tile
NBLK = 512        # matmul moving free dim
ND = D // 128     # 8 d-tiles
NP = P // 128     # 8 p-tiles
NKT = SEQ // KTL  # 16 k-tiles
NQT = 8           # q-tiles per core
SCALE = 1.0 / float(np.sqrt(P))
NEG = -1e9


def _extent(i):
    return 16 - 2 * i


def _chunks(width):
    out = []
    w = width
    while w >= NBLK:
        out.append(NBLK)
        w -= NBLK
    if w:
        assert w == 256, w
        out.append(256)
    return out


def build_program():
    nc = bacc.Bacc("TRN2", target_bir_lowering=False)

    xT = nc.dram_tensor("xT", [D, SEQ], F16, kind="ExternalInput")
    xq_in = nc.dram_tensor("xqcols", [D, NQT * QT], F16, kind="ExternalInput")
    WqT = nc.dram_tensor("WqT", [D, P], F16, kind="ExternalInput")
    WkT = nc.dram_tensor("WkT", [D, P], F16, kind="ExternalInput")
    WvT = nc.dram_tensor("WvT", [D, P], F16, kind="ExternalInput")
    mask = nc.dram_tensor("mask", [QT, 256], F32, kind="ExternalInput")
    ident_in = nc.dram_tensor("ident", [128, 128], F16, kind="ExternalInput")
    out = nc.dram_tensor("out", [NQT * QT, P], F32, kind="ExternalOutput")

    qt_scr = nc.dram_tensor("qt_scr", [P, NQT * QT], F16)

    # DRAM views with the 128-partition dim innermost-first for tiled DMA
    xT_r = xT.rearrange("(dt dp) s -> dp dt s", dp=128)          # [128, 8, SEQ]
    xq_r = xq_in.rearrange("(dt dp) q -> dp dt q", dp=128)       # [128, 8, 1024]
    qt_scr_r = qt_scr.rearrange("(pt pp) q -> pp pt q", pp=128)  # [128, 8, 1024]

    with tile.TileContext(nc) as tc:
        with tc.tile_pool(name="resident", bufs=1) as resident:
            # --- resident tensors (live the whole kernel) ---
            kt_sb = resident.tile([128, NP, SEQ], F16)     # K.T  [p, k]
            v_sb = resident.tile([128, NKT, P], F16)       # V    [k, p]
            mask_sb = resident.tile([QT, 256], F32)
            ident = resident.tile([128, 128], F16)
            nc.sync.dma_start(out=mask_sb, in_=mask[:, :])
            nc.sync.dma_start(out=ident, in_=ident_in[:, :])

            _phase0(nc, tc, kt_sb, v_sb, WqT, WkT, WvT, xT_r, xq_r, qt_scr_r)
            _phase1(nc, tc, kt_sb, v_sb, mask_sb, ident, qt_scr_r, out)

    nc.compile()
    return nc


def _phase0(nc, tc, kt_sb, v_sb, WqT, WkT, WvT, xT_r, xq_r, qt_scr_r):
    with (
        tc.tile_pool(name="wpool", bufs=1) as wpool,
        tc.tile_pool(name="xstage", bufs=1) as xstage,
        tc.tile_pool(name="stage", bufs=3) as stage,
        tc.tile_pool(name="p0psum", bufs=4, space="PSUM") as p0psum,
    ):
            # --- phase 0a: Q.T -> DRAM scratch (i-ordered columns) ---
            for wh in range(2):  # W column halves (p-tiles 4*wh..4*wh+3)
                wq_h = wpool.tile([128, ND, 4 * 128], F16, tag="w")
                nc.sync.dma_start(
                    out=wq_h,
                    in_=WqT.rearrange("(dt dp) p -> dp dt p", dp=128)[
                        :, :, wh * 512:(wh + 1) * 512],
                )
                for qg in range(2):
                    xq = xstage.tile([128, ND, NBLK], F16, tag="xq")
                    nc.sync.dma_start(
                        out=xq,
                        in_=xq_r[:, :, qg * 512:(qg + 1) * 512],
                    )
                    for ptl in range(4):
                        pt = 4 * wh + ptl
                        ps = p0psum.tile([128, NBLK], F32, tag="p0")
                        for d in range(ND):
                            nc.tensor.matmul(
                                ps,
                                wq_h[:, d, ptl * 128:(ptl + 1) * 128],
                                xq[:, d, :],
                                start=(d == 0),
                                stop=(d == ND - 1),
                            )
                        st = stage.tile([128, NBLK], F16, tag="qst")
                        nc.scalar.copy(st, ps)
                        nc.sync.dma_start(
                            out=qt_scr_r[:, pt, qg * 512:(qg + 1) * 512
                                         ],
                            in_=st,
                        )

            # --- phase 0b+0c: K.T and V (fused over k-blocks, W halves) ---
            for h in range(2):
                wk_h = wpool.tile([128, ND, 4 * 128], F16, tag="w")
                nc.sync.dma_start(
                    out=wk_h,
                    in_=WkT.rearrange("(dt dp) p -> dp dt p", dp=128)[
                        :, :, h * 512:(h + 1) * 512],
                )
                wv_h = wpool.tile([128, ND, NBLK], F16, tag="wv")
                nc.sync.dma_start(
                    out=wv_h,
                    in_=WvT.rearrange("(dt dp) p -> dp dt p", dp=128)[
                        :, :, h * 512:(h + 1) * 512],
                )
                for kb in range(SEQ // NBLK):
                    xk = xstage.tile([128, ND, NBLK], F16, tag="xq")
                    nc.sync.dma_start(
                        out=xk,
                        in_=xT_r[:, :, kb * NBLK:(kb + 1) * NBLK],
                    )
                    # K.T rows for p-tiles in this half
                    for ptl in range(4):
                        pt = 4 * h + ptl
                        ps = p0psum.tile([128, NBLK], F32, tag="p0")
                        for d in range(ND):
                            nc.tensor.matmul(
                                ps,
                                wk_h[:, d, ptl * 128:(ptl + 1) * 128],
                                xk[:, d, :],
                                start=(d == 0),
                                stop=(d == ND - 1),
                            )
                        nc.scalar.copy(kt_sb[:, pt, kb * NBLK:(kb + 1) * NBLK], ps)
                    # V columns for this half: 4 k-tiles per block
                    for j in range(NBLK // KTL):
                        ktile = kb * (NBLK // KTL) + j
                        ps = p0psum.tile([128, NBLK], F32, tag="p0")
                        for d in range(ND):
                            nc.tensor.matmul(
                                ps,
                                xk[:, d, j * 128:(j + 1) * 128],
                                wv_h[:, d, :],
                                start=(d == 0),
                                stop=(d == ND - 1),
                            )
                        nc.vector.tensor_copy(
                            v_sb[:, ktile, h * NBLK:(h + 1) * NBLK], ps
                        )


def _phase1(nc, tc, kt_sb, v_sb, mask_sb, ident, qt_scr_r, out):
    with (
        tc.tile_pool(name="qtp", bufs=2) as qtp,
        tc.tile_pool(name="wrow", bufs=2) as wrow,
        tc.tile_pool(name="small", bufs=4) as small,
        tc.tile_pool(name="outp", bufs=2) as outp,
        tc.tile_pool(name="spsum", bufs=3, space="PSUM") as spsum,
        tc.tile_pool(name="s2psum", bufs=1, space="PSUM") as s2psum,
        tc.tile_pool(name="zpsum", bufs=2, space="PSUM") as zpsum,
        tc.tile_pool(name="tpsum", bufs=2, space="PSUM") as tpsum,
    ):
            for i in range(NQT):
                ext = _extent(i)
                width = ext * KTL
                chunks = _chunks(width)

                qt_sb = qtp.tile([128, NP, QT], F16, tag="qt")
                nc.sync.dma_start(
                    out=qt_sb,
                    in_=qt_scr_r[:, :, i * QT:(i + 1) * QT],
                )

                # scores chunks in PSUM
                s_ps = []
                off = 0
                for cw in chunks:
                    pool = spsum if cw == NBLK else s2psum
                    ps = pool.tile([QT, cw], F32, tag=f"s{cw}")
                    for pt in range(NP):
                        nc.tensor.matmul(
                            ps,
                            qt_sb[:, pt, :],
                            kt_sb[:, pt, off:off + cw],
                            start=(pt == 0),
                            stop=(pt == NP - 1),
                        )
                    s_ps.append((ps, off, cw))
                    off += cw

                # additive causal mask on the last 256 columns of the row
                last_ps, _, last_w = s_ps[-1]
                nc.vector.tensor_add(
                    last_ps[:, last_w - 256:last_w],
                    last_ps[:, last_w - 256:last_w],
                    mask_sb,
                )

                # exp((s + m) * scale) -> SBUF weights row; row-sums per chunk
                w_sb = wrow.tile([QT, width], F16, tag="w")
                lparts = small.tile([QT, len(chunks)], F32, tag="lp")
                for ci, (ps, off_c, cw) in enumerate(s_ps):
                    nc.scalar.activation(
                        w_sb[:, off_c:off_c + cw],
                        ps,
                        mybir.ActivationFunctionType.Exp,
                        scale=SCALE,
                        accum_out=lparts[:, ci:ci + 1],
                    )

                lsum = small.tile([QT, 1], F32, tag="ls")
                nc.vector.reduce_sum(lsum, lparts, axis=mybir.AxisListType.X)
                rl = small.tile([QT, 1], F32, tag="rl")
                nc.vector.reciprocal(rl, lsum)

                # AV: transpose each weight block on PE, accumulate Z
                z0 = zpsum.tile([QT, NBLK], F32, tag="z")
                z1 = zpsum.tile([QT, NBLK], F32, tag="z")
                for kt in range(ext):
                    tp = tpsum.tile([128, 128], F16, tag="tp")
                    nc.tensor.transpose(
                        tp, w_sb[:, kt * 128:(kt + 1) * 128], ident
                    )
                    wT = small.tile([128, 128], F16, tag="wT")
                    nc.vector.tensor_copy(wT, tp)
                    nc.tensor.matmul(
                        z0, wT, v_sb[:, kt, 0:NBLK],
                        start=(kt == 0), stop=(kt == ext - 1),
                    )
                    nc.tensor.matmul(
                        z1, wT, v_sb[:, kt, NBLK:P],
                        start=(kt == 0), stop=(kt == ext - 1),
                    )

                o_sb = outp.tile([QT, P], F32, tag="o")
                nc.vector.tensor_scalar_mul(o_sb[:, 0:NBLK], z0, rl)
                nc.vector.tensor_scalar_mul(o_sb[:, NBLK:P], z1, rl)
                nc.sync.dma_start(out=out[i * QT:(i + 1) * QT, :], in_=o_sb)


def _tiles_for_core(c):
    """Global 128-row query-tile indices, in program order i=0..7."""
    return [(15 - 2 * i) if c < 4 else (14 - 2 * i) for i in range(NQT)]


def _host_prep(inputs, Wq, Wk, Wv):
    x = np.asarray(inputs, dtype=np.float32)
    WqT = np.ascontiguousarray(np.asarray(Wq, dtype=np.float32).T.astype(np.float16))
    WkT = np.ascontiguousarray(np.asarray(Wk, dtype=np.float32).T.astype(np.float16))
    WvT = np.ascontiguousarray(np.asarray(Wv, dtype=np.float32).T.astype(np.float16))

    qi = np.arange(QT)[:, None]
    ki = np.arange(128)[None, :]
    tri = np.where(qi >= ki, 0.0, NEG).astype(np.float32)
    mask_hi = np.concatenate([np.zeros((QT, 128), np.float32), tri], axis=1)
    mask_lo = np.concatenate([tri, np.full((QT, 128), NEG, np.float32)], axis=1)

    in_maps = []
    xT_cache = {}
    for c in range(N_CORES):
        b = c % 4
        if b not in xT_cache:
            xT_cache[b] = np.ascontiguousarray(x[b].T.astype(np.float16))  # [D, SEQ]
        xTb = xT_cache[b]
        cols = np.concatenate(
            [xTb[:, t * QT:(t + 1) * QT] for t in _tiles_for_core(c)], axis=1
        )
        in_maps.append({
            "xT": xTb,
            "xqcols": np.ascontiguousarray(cols),
            "WqT": WqT,
            "WkT": WkT,
            "WvT": WvT,
            "mask": mask_hi if c < 4 else mask_lo,
            "ident": np.eye(128, dtype=np.float16),
        })
    return in_maps


def _host_gather(results):
    Z = np.empty((BATCH, SEQ, P), dtype=np.float32)
    for c in range(N_CORES):
        b = c % 4
        o = results[c]["out"]
        for i, t in enumerate(_tiles_for_core(c)):
            Z[b, t * QT:(t + 1) * QT, :] = o[i * QT:(i + 1) * QT, :]
    return Z


_NC_CACHE = None


def kernel(inputs, Wq, Wk, Wv):
    global _NC_CACHE
    if _NC_CACHE is None:
        _NC_CACHE = build_program()
    in_maps = _host_prep(inputs, Wq, Wk, Wv)
    res = run_bass_kernel_spmd(_NC_CACHE, in_maps, list(range(N_CORES)))
    return _host_gather(res.results)


# revision 11
# speedup vs baseline: 1.5405x; 1.2972x over previous
"""Causal self-attention (single head) on 8 Trainium2 NeuronCores.

Sharding: 8 cores = 4 batches x 2 query-tile parity sets. Core c handles
batch (c % 4). Cores 0-3 take query tiles t in {15,13,...,1} (128 rows
each), cores 4-7 take t in {14,12,...,0}. Program iteration i=0..7 uses a
fixed causal extent E(i) = 16-2i k-tiles, so a single SPMD program serves
all cores; even-parity cores waste one fully-masked k-tile per iteration.

Host passes x.T (plus the core's own query columns pre-gathered) and W.T
per core so the device never transposes inputs; matmuls run as float32r.
Softmax skips max-subtraction (scores/32 stay in a safe exp range) and
gets row sums free via the activation accum_out.
"""

import sys

for _p in ("/opt/trn_rl_repo", "/root/.axon_site/_ro/trn_rl_repo"):
    if _p not in sys.path:
        sys.path.append(_p)

import numpy as np

import concourse.bass as bass  # noqa: F401
import concourse.mybir as mybir
import concourse.tile as tile
from concourse import bacc
from concourse.bass_utils import run_bass_kernel_spmd

F32 = mybir.dt.float32
F16 = mybir.dt.float16

BATCH, SEQ, D, P = 4, 2048, 1024, 1024
N_CORES = 8
QT = 128          # query tile rows
KTL = 128         # key tile
NBLK = 512        # matmul moving free dim
ND = D // 128     # 8 d-tiles
NP = P // 128     # 8 p-tiles
NKT = SEQ // KTL  # 16 k-tiles
NQT = 8           # q-tiles per core
SCALE = 1.0 / float(np.sqrt(P))
NEG = -1e9


def _extent(i):
    return 16 - 2 * i


def _chunks(width):
    out = []
    w = width
    while w >= NBLK:
        out.append(NBLK)
        w -= NBLK
    if w:
        assert w == 256, w
        out.append(256)
    return out


def build_program():
    nc = bacc.Bacc("TRN2", target_bir_lowering=False)

    xT = nc.dram_tensor("xT", [D, SEQ], F16, kind="ExternalInput")
    xq_in = nc.dram_tensor("xqcols", [D, NQT * QT], F16, kind="ExternalInput")
    WqT = nc.dram_tensor("WqT", [D, P], F16, kind="ExternalInput")
    WkT = nc.dram_tensor("WkT", [D, P], F16, kind="ExternalInput")
    WvT = nc.dram_tensor("WvT", [D, P], F16, kind="ExternalInput")
    mask = nc.dram_tensor("mask", [QT, 256], F32, kind="ExternalInput")
    ident_in = nc.dram_tensor("ident", [128, 128], F16, kind="ExternalInput")
    out = nc.dram_tensor("out", [NQT * QT, P], F32, kind="ExternalOutput")

    qt_scr = nc.dram_tensor("qt_scr", [P, NQT * QT], F16)

    # DRAM views with the 128-partition dim innermost-first for tiled DMA
    xT_r = xT.rearrange("(dt dp) s -> dp dt s", dp=128)          # [128, 8, SEQ]
    xq_r = xq_in.rearrange("(dt dp) q -> dp dt q", dp=128)       # [128, 8, 1024]
    qt_scr_r = qt_scr.rearrange("(pt pp) q -> pp pt q", pp=128)  # [128, 8, 1024]

    with tile.TileContext(nc) as tc:
        with tc.tile_pool(name="resident", bufs=1) as resident:
            # --- resident tensors (live the whole kernel) ---
            kt_sb = resident.tile([128, NP, SEQ], F16)     # K.T  [p, k]
            v_sb = resident.tile([128, NKT, P], F16)       # V    [k, p]
            mask_sb = resident.tile([QT, 256], F32)
            ident = resident.tile([128, 128], F16)
            nc.sync.dma_start(out=mask_sb, in_=mask[:, :])
            nc.sync.dma_start(out=ident, in_=ident_in[:, :])

            _phase0(nc, tc, kt_sb, v_sb, WqT, WkT, WvT, xT_r, xq_r, qt_scr_r)
            _phase1(nc, tc, kt_sb, v_sb, mask_sb, ident, qt_scr_r, out)

    nc.compile()
    return nc


def _phase0(nc, tc, kt_sb, v_sb, WqT, WkT, WvT, xT_r, xq_r, qt_scr_r):
    with (
        tc.tile_pool(name="wpool", bufs=2) as wpool,
        tc.tile_pool(name="xstage", bufs=3) as xstage,
        tc.tile_pool(name="stage", bufs=3) as stage,
        tc.tile_pool(name="p0psum", bufs=4, space="PSUM") as p0psum,
    ):
            # --- phase 0a: Q.T -> DRAM scratch (i-ordered columns) ---
            for wh in range(2):  # W column halves (p-tiles 4*wh..4*wh+3)
                wq_h = wpool.tile([128, ND, 4 * 128], F16, tag="w")
                nc.sync.dma_start(
                    out=wq_h,
                    in_=WqT.rearrange("(dt dp) p -> dp dt p", dp=128)[
                        :, :, wh * 512:(wh + 1) * 512],
                )
                for qg in range(2):
                    xq = xstage.tile([128, ND, NBLK], F16, tag="xq")
                    nc.sync.dma_start(
                        out=xq,
                        in_=xq_r[:, :, qg * 512:(qg + 1) * 512],
                    )
                    for ptl in range(4):
                        pt = 4 * wh + ptl
                        ps = p0psum.tile([128, NBLK], F32, tag="p0")
                        for d in range(ND):
                            nc.tensor.matmul(
                                ps,
                                wq_h[:, d, ptl * 128:(ptl + 1) * 128],
                                xq[:, d, :],
                                start=(d == 0),
                                stop=(d == ND - 1),
                            )
                        st = stage.tile([128, NBLK], F16, tag="qst")
                        nc.scalar.copy(st, ps)
                        nc.sync.dma_start(
                            out=qt_scr_r[:, pt, qg * 512:(qg + 1) * 512
                                         ],
                            in_=st,
                        )

            # --- phase 0b+0c: K.T and V (fused over k-blocks, W halves) ---
            for h in range(2):
                wk_h = wpool.tile([128, ND, 4 * 128], F16, tag="w")
                nc.sync.dma_start(
                    out=wk_h,
                    in_=WkT.rearrange("(dt dp) p -> dp dt p", dp=128)[
                        :, :, h * 512:(h + 1) * 512],
                )
                wv_h = wpool.tile([128, ND, NBLK], F16, tag="wv")
                nc.sync.dma_start(
                    out=wv_h,
                    in_=WvT.rearrange("(dt dp) p -> dp dt p", dp=128)[
                        :, :, h * 512:(h + 1) * 512],
                )
                for kb in range(SEQ // NBLK):
                    xk = xstage.tile([128, ND, NBLK], F16, tag="xq")
                    nc.sync.dma_start(
                        out=xk,
                        in_=xT_r[:, :, kb * NBLK:(kb + 1) * NBLK],
                    )
                    # K.T rows for p-tiles in this half
                    for ptl in range(4):
                        pt = 4 * h + ptl
                        ps = p0psum.tile([128, NBLK], F32, tag="p0")
                        for d in range(ND):
                            nc.tensor.matmul(
                                ps,
                                wk_h[:, d, ptl * 128:(ptl + 1) * 128],
                                xk[:, d, :],
                                start=(d == 0),
                                stop=(d == ND - 1),
                            )
                        nc.scalar.copy(kt_sb[:, pt, kb * NBLK:(kb + 1) * NBLK], ps)
                    # V columns for this half: 4 k-tiles per block
                    for j in range(NBLK // KTL):
                        ktile = kb * (NBLK // KTL) + j
                        ps = p0psum.tile([128, NBLK], F32, tag="p0")
                        for d in range(ND):
                            nc.tensor.matmul(
                                ps,
                                xk[:, d, j * 128:(j + 1) * 128],
                                wv_h[:, d, :],
                                start=(d == 0),
                                stop=(d == ND - 1),
                            )
                        nc.vector.tensor_copy(
                            v_sb[:, ktile, h * NBLK:(h + 1) * NBLK], ps
                        )


def _phase1(nc, tc, kt_sb, v_sb, mask_sb, ident, qt_scr_r, out):
    with (
        tc.tile_pool(name="qtp", bufs=3) as qtp,
        tc.tile_pool(name="wrow", bufs=3) as wrow,
        tc.tile_pool(name="small", bufs=6) as small,
        tc.tile_pool(name="outp", bufs=2) as outp,
        tc.tile_pool(name="spsum", bufs=3, space="PSUM") as spsum,
        tc.tile_pool(name="s2psum", bufs=1, space="PSUM") as s2psum,
        tc.tile_pool(name="zpsum", bufs=3, space="PSUM") as zpsum,
        tc.tile_pool(name="tpsum", bufs=1, space="PSUM") as tpsum,
    ):
            for i in range(NQT):
                ext = _extent(i)
                width = ext * KTL
                chunks = _chunks(width)

                qt_sb = qtp.tile([128, NP, QT], F16, tag="qt")
                nc.sync.dma_start(
                    out=qt_sb,
                    in_=qt_scr_r[:, :, i * QT:(i + 1) * QT],
                )

                # scores chunks in PSUM
                s_ps = []
                off = 0
                for cw in chunks:
                    pool = spsum if cw == NBLK else s2psum
                    ps = pool.tile([QT, cw], F32, tag=f"s{cw}")
                    for pt in range(NP):
                        nc.tensor.matmul(
                            ps,
                            qt_sb[:, pt, :],
                            kt_sb[:, pt, off:off + cw],
                            start=(pt == 0),
                            stop=(pt == NP - 1),
                        )
                    s_ps.append((ps, off, cw))
                    off += cw

                # additive causal mask on the last 256 columns of the row
                last_ps, _, last_w = s_ps[-1]
                nc.vector.tensor_add(
                    last_ps[:, last_w - 256:last_w],
                    last_ps[:, last_w - 256:last_w],
                    mask_sb,
                )

                # exp((s + m) * scale) -> SBUF weights row; row-sums per chunk
                w_sb = wrow.tile([QT, width], F16, tag="w")
                lparts = small.tile([QT, len(chunks)], F32, tag="lp")
                for ci, (ps, off_c, cw) in enumerate(s_ps):
                    nc.scalar.activation(
                        w_sb[:, off_c:off_c + cw],
                        ps,
                        mybir.ActivationFunctionType.Exp,
                        scale=SCALE,
                        accum_out=lparts[:, ci:ci + 1],
                    )

                lsum = small.tile([QT, 1], F32, tag="ls")
                nc.vector.reduce_sum(lsum, lparts, axis=mybir.AxisListType.X)
                rl = small.tile([QT, 1], F32, tag="rl")
                nc.vector.reciprocal(rl, lsum)

                # AV: transpose each weight block on PE, accumulate Z
                z0 = zpsum.tile([QT, NBLK], F32, tag="z")
                z1 = zpsum.tile([QT, NBLK], F32, tag="z")
                for kt in range(ext):
                    tp = tpsum.tile([128, 128], F16, tag="tp")
                    nc.tensor.transpose(
                        tp, w_sb[:, kt * 128:(kt + 1) * 128], ident
                    )
                    wT = small.tile([128, 128], F16, tag="wT")
                    nc.vector.tensor_copy(wT, tp)
                    nc.tensor.matmul(
                        z0, wT, v_sb[:, kt, 0:NBLK],
                        start=(kt == 0), stop=(kt == ext - 1),
                    )
                    nc.tensor.matmul(
                        z1, wT, v_sb[:, kt, NBLK:P],
                        start=(kt == 0), stop=(kt == ext - 1),
                    )

                o_sb = outp.tile([QT, P], F32, tag="o")
                nc.vector.tensor_scalar_mul(o_sb[:, 0:NBLK], z0, rl)
                nc.vector.tensor_scalar_mul(o_sb[:, NBLK:P], z1, rl)
                nc.sync.dma_start(out=out[i * QT:(i + 1) * QT, :], in_=o_sb)


def _tiles_for_core(c):
    """Global 128-row query-tile indices, in program order i=0..7."""
    return [(15 - 2 * i) if c < 4 else (14 - 2 * i) for i in range(NQT)]


def _host_prep(inputs, Wq, Wk, Wv):
    x = np.asarray(inputs, dtype=np.float32)
    WqT = np.ascontiguousarray(np.asarray(Wq, dtype=np.float32).T.astype(np.float16))
    WkT = np.ascontiguousarray(np.asarray(Wk, dtype=np.float32).T.astype(np.float16))
    WvT = np.ascontiguousarray(np.asarray(Wv, dtype=np.float32).T.astype(np.float16))

    qi = np.arange(QT)[:, None]
    ki = np.arange(128)[None, :]
    tri = np.where(qi >= ki, 0.0, NEG).astype(np.float32)
    mask_hi = np.concatenate([np.zeros((QT, 128), np.float32), tri], axis=1)
    mask_lo = np.concatenate([tri, np.full((QT, 128), NEG, np.float32)], axis=1)

    in_maps = []
    xT_cache = {}
    for c in range(N_CORES):
        b = c % 4
        if b not in xT_cache:
            xT_cache[b] = np.ascontiguousarray(x[b].T.astype(np.float16))  # [D, SEQ]
        xTb = xT_cache[b]
        cols = np.concatenate(
            [xTb[:, t * QT:(t + 1) * QT] for t in _tiles_for_core(c)], axis=1
        )
        in_maps.append({
            "xT": xTb,
            "xqcols": np.ascontiguousarray(cols),
            "WqT": WqT,
            "WkT": WkT,
            "WvT": WvT,
            "mask": mask_hi if c < 4 else mask_lo,
            "ident": np.eye(128, dtype=np.float16),
        })
    return in_maps


def _host_gather(results):
    Z = np.empty((BATCH, SEQ, P), dtype=np.float32)
    for c in range(N_CORES):
        b = c % 4
        o = results[c]["out"]
        for i, t in enumerate(_tiles_for_core(c)):
            Z[b, t * QT:(t + 1) * QT, :] = o[i * QT:(i + 1) * QT, :]
    return Z


_NC_CACHE = None


def kernel(inputs, Wq, Wk, Wv):
    global _NC_CACHE
    if _NC_CACHE is None:
        _NC_CACHE = build_program()
    in_maps = _host_prep(inputs, Wq, Wk, Wv)
    res = run_bass_kernel_spmd(_NC_CACHE, in_maps, list(range(N_CORES)))
    return _host_gather(res.results)
